# revision 46
# baseline (speedup 1.0000x reference)
"""Trainium2 Bass kernel for GNN message-passing attention block.

Sharding: core c handles batch c//4 and query block (c%4)*256..+256.
Each core computes GroupNorm + K/V projections over its batch's full
n=1024 nodes (attend_idx may reference any node), but only its own 256
queries' Q/scores/softmax/AV/output.

The sparse gather/scatter collapses into a dense multiplicity mask
M[j,q] (host-precomputed): duplicate neighbor indices share a score, so
their combined softmax weight is M * exp(s) / Z.  Z comes free from an
appended ones-column in the V^T stationary.  The K bias is dropped
(softmax-invariant); the Q bias rides the PSUM->SBUF Identity cast.

Schedule notes (from NTFF trace iterations; baseline 126us -> ~75us):
- DMA issues cost ~600ns of issuing-engine time each: tensors are
  consolidated to one DMA per tensor (host pre-transposes to
  partition-major), split across scalar/gpsimd/sync queues by need-time.
- exp/mask work in [128,1024] ops (jc-pairs) to amortize fixed costs;
  scores interleave K/V-projection and AV fillers so the PE (the pacing
  engine: ~230 matmuls at a power-throttled ~1.2-1.8GHz) never idles
  while ACT streams exps.
- AV groups 0/1 run as fillers inside the last score block, so only
  group 3's AV+normalize trails the exp stream.
- All PSUM evacuation (K/V/Q casts, Z rows, osb) must stay off GPSIMD
  (no PSUM access) and off ACT mid-stream (delays exp); DVE carries it.
"""

import sys

if "/opt/trn_rl_repo" not in sys.path:
    sys.path.insert(0, "/opt/trn_rl_repo")

import numpy as np

import concourse.bacc as bacc
import concourse.mybir as mybir
import concourse.tile as tile
from concourse import bass_utils
from contextlib import ExitStack

F32 = mybir.dt.float32
BF16 = mybir.dt.bfloat16
AF = mybir.ActivationFunctionType
ALU = mybir.AluOpType

B, C, N, K, H, DH, NG = 2, 512, 1024, 64, 8, 64, 32
NQ = 256            # queries per core
NCHUNK = C // 128   # 4 channel chunks
NJC = 8             # key-node chunks of 128
NJP = 4             # jc pairs
EPS = 1e-6

_CACHE = {}


def _emit(tc, nc, t):
    ctx = t["ctx"]
    P = 128

    wpool = ctx.enter_context(tc.tile_pool(name="weights", bufs=1))
    cpool = ctx.enter_context(tc.tile_pool(name="consts", bufs=1))
    xpool = ctx.enter_context(tc.tile_pool(name="x", bufs=1))
    hpool = ctx.enter_context(tc.tile_pool(name="h", bufs=1))
    kvpool = ctx.enter_context(tc.tile_pool(name="kv", bufs=1))
    apool = ctx.enter_context(tc.tile_pool(name="attn", bufs=1))
    spool = ctx.enter_context(tc.tile_pool(name="scratch", bufs=2))
    smallp = ctx.enter_context(tc.tile_pool(name="small", bufs=1))
    opool = ctx.enter_context(tc.tile_pool(name="out", bufs=1))
    ypool = ctx.enter_context(tc.tile_pool(name="yp", bufs=2))
    # PSUM: pp 2x[128,512] + ps 2x[128,1024] + po 2x[65,512] = 8 banks
    pp = ctx.enter_context(tc.tile_pool(name="psum", bufs=2, space="PSUM"))
    ps_pool = ctx.enter_context(tc.tile_pool(name="psum_s", bufs=2, space="PSUM"))
    po_pool = ctx.enter_context(tc.tile_pool(name="psum_o", bufs=2, space="PSUM"))

    # ---- DMA issues, one per tensor, ordered by first-use ----
    gbwd = cpool.tile([8, P], F32)
    consts = cpool.tile([P, 12], F32)      # gamma|beta|bo_eff (4 cols each)
    bq4 = cpool.tile([P, 4], F32)
    gfwd = cpool.tile([P, 8], F32)
    nc.scalar.dma_start(gbwd[:], t["gbwd"][:, :])     # first: feeds PE warmup
    nc.scalar.dma_start(consts[:], t["consts"][:, :])
    nc.scalar.dma_start(bq4[:], t["bq4"][:, :])
    nc.scalar.dma_start(gfwd[:], t["gfwd"][:, :])

    xsb = [xpool.tile([P, N], BF16, tag=f"x{m}", name=f"x{m}") for m in range(NCHUNK)]
    xqs = xpool.tile([P, NCHUNK, NQ], F32)
    wq = wpool.tile([P, NCHUNK, 512], BF16, tag="wq", name="wq")
    wk = wpool.tile([P, NCHUNK, 512], BF16, tag="wk", name="wk")
    wv = wpool.tile([P, NCHUNK, 512], BF16, tag="wv", name="wv")
    wo = wpool.tile([P, NCHUNK, 512], BF16, tag="wo", name="wo")
    msb = cpool.tile([P, NJP, 2, 512], BF16, tag="msb", name="msb")

    for m in range(NCHUNK):
        nc.gpsimd.dma_start(xsb[m][:], t["xb"][m, :, :])
    nc.gpsimd.dma_start(wq[:], t["wqT"][:, :, :])
    nc.sync.dma_start(xqs[:], t["xq"][:, :, :])
    nc.sync.dma_start(wk[:], t["wkT"][:, :, :])
    xqb = xpool.tile([P, NCHUNK, NQ + 1], BF16, tag="xqb", name="xqb")
    nc.sync.dma_start(xqb[:, :, 0:NQ], t["xqb"][:, :, :])
    nc.gpsimd.dma_start(wv[:], t["wvT"][:, :, :])
    nc.gpsimd.dma_start(wo[:], t["woT"][:, :, :])
    nc.gpsimd.dma_start(msb[:], t["mmask"][:, :, :, :])

    ones64 = cpool.tile([1, DH], BF16)
    nc.gpsimd.memset(ones64[:], 1.0)
    expwarm = cpool.tile([1, 2], F32)
    nc.gpsimd.memset(expwarm[:], 0.0)

    # ---- GroupNorm statistics (per-partition bn_stats -> PE group aggr) ----
    ssq = smallp.tile([P, 2, 4], F32)    # [., 0, m]=mean_p, [., 1, m]=E[x^2]_p
    sqt = smallp.tile([P, 4], F32)
    bnraw = smallp.tile([P, 2, 6], F32)
    for m in range(NCHUNK):
        for tblk in range(2):
            nc.vector.bn_stats(bnraw[:, tblk, :],
                               xsb[m][:, tblk * 512:(tblk + 1) * 512])
        # (mean, var) -> strided cols (m, 4+m) of ssq
        nc.vector.bn_aggr(ssq[:, :, m], bnraw[:])
    # E[x^2] = var + mean^2 for all four chunks at once
    nc.vector.tensor_tensor(sqt[:], ssq[:, 0, :], ssq[:, 0, :], ALU.mult)
    nc.vector.tensor_tensor(ssq[:, 1, :], sqt[:], ssq[:, 1, :], ALU.add)
    gs = pp.tile([8, 8], F32, tag="mm", name="gs")
    nc.tensor.matmul(gs[:], gfwd[:], ssq[:], start=True, stop=True)
    mu = smallp.tile([8, 8], F32)
    nc.scalar.activation(mu[:], gs[:], AF.Copy, scale=1.0 / 16.0)
    var = smallp.tile([8, 4], F32)
    nc.vector.tensor_tensor(var[:], mu[:, 0:4], mu[:, 0:4], ALU.mult)
    nc.vector.tensor_tensor(var[:], mu[:, 4:8], var[:], ALU.subtract)
    sd = smallp.tile([8, 4], F32)
    nc.vector.tensor_scalar_add(sd[:], var[:], EPS)
    sdq = smallp.tile([8, 4], F32)
    nc.scalar.activation(sdq[:], sd[:], AF.Sqrt)
    # Warm the exp act-table now: exp_and_others also holds identity/copy,
    # so the Identity/Copy casts below cause no further table loads.
    nc.scalar.activation(expwarm[:], expwarm[:], AF.Exp)
    rs = smallp.tile([8, 4], F32)
    nc.vector.reciprocal(rs[:], sdq[:])
    bc = pp.tile([P, 8], F32, tag="mm", name="bc")
    nc.tensor.matmul(bc[:, 0:4], gbwd[:], mu[:, 0:4], start=True, stop=True)
    nc.tensor.matmul(bc[:, 4:8], gbwd[:], rs[:], start=True, stop=True)
    ga = smallp.tile([P, 4], F32)
    gb = smallp.tile([P, 4], F32)
    nc.vector.tensor_tensor(ga[:], consts[:, 0:4], bc[:, 4:8], ALU.mult)
    nc.vector.tensor_tensor(gb[:], bc[:, 0:4], ga[:], ALU.mult)
    nc.vector.tensor_tensor(gb[:], consts[:, 4:8], gb[:], ALU.subtract)

    # ---- Fold GN into the K/Q weights: k = Wk(ga*x + gb) = (Wk diag(ga)) x
    # + Wk gb.  The K bias term is softmax-invariant (drops); Q's comes out
    # of a gb column appended to the moving operand.  K/Q then consume raw
    # x and skip the h stage entirely; V keeps the h path (x is its
    # stationary operand, so the gb term is not a per-channel constant
    # there) ----
    for m in range(NCHUNK):
        nc.vector.tensor_scalar(wk[:, m, :], wk[:, m, :], ga[:, m:m + 1],
                                None, ALU.mult)
    for m in range(NCHUNK):
        nc.vector.tensor_copy(xqb[:, m, NQ:NQ + 1], gb[:, m:m + 1])
        nc.vector.tensor_scalar(wq[:, m, :], wq[:, m, :], ga[:, m:m + 1],
                                None, ALU.mult)
    hsb = [hpool.tile([P, N], BF16, tag=f"h{m}", name=f"h{m}") for m in range(NCHUNK)]

    # ---- K projection (bias dropped), V projection.  Odd head halves get
    # partition-0 copies via SBUF DMA (matmul operands at partition offset
    # 64 crash hardware); K group 0 runs first so its odd-half DMA hides
    # under the Q projection ----
    qsb = kvpool.tile([P, NCHUNK, NQ], BF16, tag="q", name="q")
    qOd = kvpool.tile([64, NCHUNK, NQ], BF16, tag="qO", name="qO")
    ksb = kvpool.tile([P, NCHUNK, N], BF16, tag="k", name="k")
    kOd = kvpool.tile([64, NCHUNK, N], BF16, tag="kO", name="kO")
    vT = [kvpool.tile([P, H, DH + 1], BF16, tag=f"vT{jc}", name=f"vT{jc}")
          for jc in range(NJC)]

    def k_nt(g, nt):
        osl = slice(g * 128, (g + 1) * 128)
        nsl = slice(nt * 512, (nt + 1) * 512)
        pk = pp.tile([P, 512], F32, tag="mm", name=f"pk{g}_{nt}")
        for ki in range(NCHUNK):
            nc.tensor.matmul(pk[:], wk[:, ki, osl], xsb[ki][:, nsl],
                             start=(ki == 0), stop=(ki == NCHUNK - 1))
        nc.vector.tensor_copy(ksb[:, g, nsl], pk[:])
        if g == 0:
            nc.vector.tensor_copy(kOd[:, 0, nsl], pk[64:128, :])
        elif nt == 1:
            nc.sync.dma_start(kOd[:, g, :], ksb[64:128, g, :])

    def v_jc(jc):
        jsl = slice(jc * 128, (jc + 1) * 128)
        pv = pp.tile([P, C], F32, tag="mm", name=f"pv{jc}")
        for ki in range(NCHUNK):
            nc.tensor.matmul(pv[:], hsb[ki][:, jsl], wv[:, ki, :],
                             start=(ki == 0), stop=(ki == NCHUNK - 1))
        nc.vector.tensor_copy(vT[jc][:, :, 0:DH],
                              pv[:].rearrange("p (h d) -> p h d", h=H))
        nc.gpsimd.memset(vT[jc][:, :, DH:DH + 1], 1.0)

    k_nt(0, 0)
    k_nt(0, 1)

    # ---- Q projection on raw x; pq column NQ accumulates Wq@gb, which
    # plus bq becomes the per-channel bias of the Identity cast ----
    bqe = smallp.tile([P, 4], F32)
    for g in range(NCHUNK):
        osl = slice(g * 128, (g + 1) * 128)
        pq = pp.tile([P, NQ + 1], F32, tag="mm", name=f"pq{g}")
        for ki in range(NCHUNK):
            nc.tensor.matmul(pq[:], wq[:, ki, osl], xqb[:, ki, :],
                             start=(ki == 0), stop=(ki == NCHUNK - 1))
        nc.vector.tensor_tensor(bqe[:, g:g + 1], pq[:, NQ:NQ + 1],
                                bq4[:, g:g + 1], ALU.add)
        nc.scalar.activation(qsb[:, g, :], pq[:, 0:NQ], AF.Identity,
                             bias=bqe[:, g:g + 1])
        if g == 0:
            nc.scalar.activation(qOd[:, 0, :], pq[64:128, 0:NQ], AF.Identity,
                                 bias=bqe[64:128, 0:1])
    nc.sync.dma_start(qOd[:, 1:4, :], qsb[64:128, 1:4, :])

    # h (for the V projection only) after the critical K0/Q evacuations in
    # the DVE queue; first V filler runs in the g0 score block
    for m in range(NCHUNK):
        nc.vector.tensor_scalar(hsb[m][:], xsb[m][:], ga[:, m:m + 1],
                                gb[:, m:m + 1], ALU.mult, ALU.add)

    # ---- scores -> exp -> mask at jc-pair granularity, with fillers ----
    asb = [apool.tile([P, 2, NCHUNK, 512], BF16, tag=f"a{jp}", name=f"a{jp}")
           for jp in range(NJP)]

    def scores_jp(g, jp):
        ps = ps_pool.tile([P, 2, 2, NQ], F32, tag="ps", name=f"ps{g}_{jp}")
        for jcin in range(2):
            jc = 2 * jp + jcin
            jsl = slice(jc * 128, (jc + 1) * 128)
            nc.tensor.matmul(ps[:, jcin, 0, :], ksb[0:64, g, jsl],
                             qsb[0:64, g, :], start=True, stop=True)
            nc.tensor.matmul(ps[:, jcin, 1, :], kOd[:, g, jsl],
                             qOd[:, g, :], start=True, stop=True)
        dst = asb[jp][:, :, g, :]
        nc.scalar.activation(dst, ps[:], AF.Exp)
        nc.vector.tensor_tensor(dst, dst, msb[:, jp, :, :], ALU.mult)

    po = [None] * NCHUNK
    zsb = [None] * NCHUNK
    zbci = [None] * NCHUNK

    def av_g(g):
        po[g] = po_pool.tile([DH + 1, 2 * NQ], F32, tag="po", name=f"po{g}")
        for h2 in range(2):
            h = 2 * g + h2
            blk = slice(h2 * NQ, (h2 + 1) * NQ)
            for jc in range(NJC):
                nc.tensor.matmul(po[g][:, blk], vT[jc][:, h, :],
                                 asb[jc // 2][:, jc % 2, g,
                                              h2 * NQ:(h2 + 1) * NQ],
                                 start=(jc == 0), stop=(jc == NJC - 1))

    def zsb_g(g):
        zsb[g] = spool.tile([1, 2 * NQ], BF16, tag="zsb", name=f"zsb{g}")
        if g == NCHUNK - 1:
            nc.vector.tensor_copy(zsb[g][:], po[g][DH:DH + 1, :])
        else:
            nc.scalar.activation(zsb[g][:], po[g][DH:DH + 1, :], AF.Copy)

    def pz_g(g):
        pz = pp.tile([DH, 2 * NQ], F32, tag="mm", name=f"pz{g}")
        nc.tensor.matmul(pz[:], ones64[:], zsb[g][:], start=True, stop=True)
        return pz

    osb = [opool.tile([P, NQ], BF16, tag=f"o{g}", name=f"o{g}") for g in range(NCHUNK)]

    def norm_g(g, pz):
        zbci[g] = spool.tile([DH, 2 * NQ], F32, tag="zbci", name=f"zbci{g}")
        nc.vector.reciprocal_approx_fast(zbci[g][:], pz[:])
        nc.vector.tensor_tensor(osb[g][0:64, :], po[g][0:DH, 0:NQ],
                                zbci[g][:, 0:NQ], ALU.mult)
        nc.vector.tensor_tensor(osb[g][64:128, :], po[g][0:DH, NQ:2 * NQ],
                                zbci[g][:, NQ:2 * NQ], ALU.mult)

    def av_z(g):
        av_g(g)
        zsb_g(g)

    fillers = {0: [lambda: k_nt(1, 0), lambda: k_nt(1, 1),
                   lambda: v_jc(0), lambda: v_jc(1)],
               1: [lambda: k_nt(2, 0), lambda: k_nt(2, 1),
                   lambda: v_jc(2), lambda: v_jc(3)],
               2: [lambda: k_nt(3, 0), lambda: k_nt(3, 1),
                   lambda: v_jc(4), lambda: v_jc(5)],
               3: [lambda: v_jc(6), lambda: v_jc(7),
                   lambda: av_z(0), lambda: av_z(1)]}
    for g in range(NCHUNK):
        todo = list(fillers[g])
        for jp in range(NJP):
            scores_jp(g, jp)
            if jp == 0 and todo:
                todo.pop(0)()
            if jp < NJP - 1 and todo:
                todo.pop(0)()
        while todo:
            todo.pop(0)()

    av_g(2)
    zsb_g(2)
    norm_g(0, pz_g(0))
    norm_g(1, pz_g(1))
    norm_g(2, pz_g(2))

    # ---- output projection + bias (bo_eff folds the V bias) + residual.
    # Chunks mo 0/1 pre-accumulate head-pairs 0-2 in the two pp banks while
    # group 3 is still normalizing; only the last contraction chunk trails
    # osb[3]. ----
    pys = {}
    for mo in (0,):
        osl = slice(mo * 128, (mo + 1) * 128)
        py = pp.tile([P, NQ], F32, tag="mm", name=f"py{mo}")
        for ki in range(NCHUNK - 1):
            nc.tensor.matmul(py[:], wo[:, ki, osl], osb[ki][:],
                             start=(ki == 0), stop=False)
        pys[mo] = py

    av_g(3)
    zsb_g(3)
    norm_g(3, pz_g(3))

    for mo in range(NCHUNK):
        osl = slice(mo * 128, (mo + 1) * 128)
        if mo in pys:
            py = pys[mo]
            nc.tensor.matmul(py[:], wo[:, NCHUNK - 1, osl], osb[NCHUNK - 1][:],
                             start=False, stop=True)
        else:
            py = pp.tile([P, NQ], F32, tag="mm", name=f"py{mo}")
            for ki in range(NCHUNK):
                nc.tensor.matmul(py[:], wo[:, ki, osl], osb[ki][:],
                                 start=(ki == 0), stop=(ki == NCHUNK - 1))
        ysb = ypool.tile([P, NQ], F32, tag="y", name=f"y{mo}")
        nc.vector.scalar_tensor_tensor(ysb[:], py[:], consts[:, 8 + mo:9 + mo],
                                       xqs[:, mo, :], ALU.add, ALU.add)
        nc.sync.dma_start(t["y"][mo, :, :], ysb[:])


def _build():
    nc = bacc.Bacc("TRN2", target_bir_lowering=False, debug=False, num_devices=8)
    t = {}
    t["xb"] = nc.dram_tensor("xb", [NCHUNK, 128, N], BF16, kind="ExternalInput").ap()
    t["xq"] = nc.dram_tensor("xq", [128, NCHUNK, NQ], F32, kind="ExternalInput").ap()
    t["xqb"] = nc.dram_tensor("xqb", [128, NCHUNK, NQ], BF16, kind="ExternalInput").ap()
    t["mmask"] = nc.dram_tensor("mmask", [128, NJP, 2, 512], BF16,
                                kind="ExternalInput").ap()
    for w in ("wqT", "wkT", "wvT", "woT"):
        t[w] = nc.dram_tensor(w, [128, NCHUNK, 512], BF16, kind="ExternalInput").ap()
    t["consts"] = nc.dram_tensor("consts", [128, 12], F32, kind="ExternalInput").ap()
    t["bq4"] = nc.dram_tensor("bq4", [128, 4], F32, kind="ExternalInput").ap()
    t["gfwd"] = nc.dram_tensor("gfwd", [128, 8], F32, kind="ExternalInput").ap()
    t["gbwd"] = nc.dram_tensor("gbwd", [8, 128], F32, kind="ExternalInput").ap()
    t["y"] = nc.dram_tensor("y", [NCHUNK, 128, NQ], F32, kind="ExternalOutput").ap()
    with tile.TileContext(nc) as tc, ExitStack() as ctx:
        t["ctx"] = ctx
        _emit(tc, nc, t)
    nc.compile()
    return nc


def _prep_inputs(inputs):
    x = np.ascontiguousarray(np.asarray(inputs["x"], dtype=np.float32))
    idx = np.asarray(inputs["attend_idx"]).astype(np.int64)
    vm = np.asarray(inputs["valid_mask"]).astype(np.float32)
    wq = np.asarray(inputs["wq"], dtype=np.float32)
    wk = np.asarray(inputs["wk"], dtype=np.float32)
    wv = np.asarray(inputs["wv"], dtype=np.float32)
    wo = np.asarray(inputs["wo"], dtype=np.float32)
    bq = np.asarray(inputs["bq"], dtype=np.float32)
    bv = np.asarray(inputs["bv"], dtype=np.float32)
    bo = np.asarray(inputs["bo"], dtype=np.float32)
    gamma = np.asarray(inputs["gn_gamma"], dtype=np.float32)
    beta = np.asarray(inputs["gn_beta"], dtype=np.float32)

    cols = np.arange(C)
    perm = (cols % DH) * H + cols // DH   # wo_perm[:, h*64+d] = wo[:, d*8+h]
    wo_perm = wo[:, perm]
    bo_eff = bo + wo_perm @ bv

    def colmajor(v):
        return np.ascontiguousarray(v.reshape(NCHUNK, 128).T)

    def wchunks(wT):
        # [C, C] -> [128, NCHUNK, 512] so one contiguous DMA fills the tile
        return np.ascontiguousarray(wT.reshape(NCHUNK, 128, C).transpose(1, 0, 2))

    consts = np.concatenate([colmajor(v) for v in (gamma, beta, bo_eff)], axis=1)
    gfwd = np.zeros((128, 8), np.float32)
    gfwd[np.arange(128), np.arange(128) // 16] = 1.0
    gbwd = np.ascontiguousarray(gfwd.T)

    from ml_dtypes import bfloat16
    x_bf = x.astype(bfloat16)
    shared = {
        "wqT": wchunks(wq.T).astype(bfloat16),
        "wkT": wchunks(wk.T).astype(bfloat16),
        "wvT": wchunks(wv.T).astype(bfloat16),
        "woT": wchunks(wo_perm.T).astype(bfloat16),
        "consts": np.ascontiguousarray(consts),
        "bq4": np.ascontiguousarray(colmajor(bq)),
        "gfwd": gfwd,
        "gbwd": gbwd,
    }
    in_maps = []
    for r in range(8):
        b = r // 4
        qs = slice((r % 4) * NQ, (r % 4 + 1) * NQ)
        Mr = np.zeros((N, NQ), np.float32)
        np.add.at(Mr, (idx[qs].ravel(), np.repeat(np.arange(NQ), K)),
                  vm[qs].ravel())
        M2 = Mr.reshape(NJC, 128, NQ)
        # [128, jp, jc-in-pair(2), dup(2)*256] head-duplicated mask blocks
        mk = np.empty((128, NJP, 2, 2, NQ), np.float32)
        for jp in range(NJP):
            for jcin in range(2):
                mk[:, jp, jcin, 0, :] = M2[2 * jp + jcin]
                mk[:, jp, jcin, 1, :] = M2[2 * jp + jcin]
        m = dict(shared)
        m["mmask"] = np.ascontiguousarray(
            mk.reshape(128, NJP, 2, 512)).astype(bfloat16)
        m["xb"] = np.ascontiguousarray(x_bf[b].reshape(NCHUNK, 128, N))
        m["xq"] = np.ascontiguousarray(
            x[b, :, qs].reshape(NCHUNK, 128, NQ).transpose(1, 0, 2))
        m["xqb"] = np.ascontiguousarray(
            x_bf[b][:, qs].reshape(NCHUNK, 128, NQ).transpose(1, 0, 2))
        in_maps.append(m)
    return in_maps


def _get_runner(n_cores=8):
    """Build (once) a cached jitted SPMD executor mirroring
    bass2jax.run_bass_via_pjrt, so repeated calls don't re-trace."""
    if "runner" in _CACHE:
        return _CACHE["runner"]
    if "nc" not in _CACHE:
        _CACHE["nc"] = _build()
    nc = _CACHE["nc"]
    import jax
    from jax.sharding import Mesh, PartitionSpec
    from jax.experimental.shard_map import shard_map
    from concourse import bass2jax
    import concourse.mybir as _mybir

    bass2jax.install_neuronx_cc_hook()
    part_name = nc.partition_id_tensor.name if nc.partition_id_tensor else None
    in_names, out_names, out_avals, zero_outs = [], [], [], []
    for alloc in nc.m.functions[0].allocations:
        if not isinstance(alloc, _mybir.MemoryLocationSet):
            continue
        name = alloc.memorylocations[0].name
        if alloc.kind == "ExternalInput":
            if name != part_name:
                in_names.append(name)
        elif alloc.kind == "ExternalOutput":
            shape = tuple(alloc.tensor_shape)
            dtype = _mybir.dt.np(alloc.dtype)
            out_names.append(name)
            out_avals.append(jax.core.ShapedArray(shape, dtype))
            zero_outs.append(np.zeros(shape, dtype))
    n_params = len(in_names)
    n_outs = len(out_avals)
    all_names = in_names + out_names
    if part_name is not None:
        all_names = all_names + [part_name]
    donate = tuple(range(n_params, n_params + n_outs))

    def _body(*args):
        operands = list(args)
        if part_name is not None:
            operands.append(bass2jax.partition_id_tensor())
        outs = bass2jax._bass_exec_p.bind(
            *operands,
            out_avals=tuple(out_avals),
            in_names=tuple(all_names),
            out_names=tuple(out_names),
            lowering_input_output_aliases=(),
            sim_require_finite=True,
            sim_require_nnan=True,
            nc=nc,
        )
        return tuple(outs)

    devices = jax.devices()[:n_cores]
    mesh = Mesh(np.asarray(devices), ("core",))
    fn = jax.jit(
        shard_map(_body, mesh=mesh,
                  in_specs=(PartitionSpec("core"),) * (n_params + n_outs),
                  out_specs=(PartitionSpec("core"),) * n_outs,
                  check_rep=False),
        donate_argnums=donate, keep_unused=True)

    def run(in_maps, device_inputs=None):
        if device_inputs is None:
            device_inputs = put_inputs(in_maps)
        zo = [np.concatenate([np.zeros_like(z)] * n_cores, axis=0)
              for z in zero_outs]
        outs = fn(*device_inputs, *zo)
        outs = [np.asarray(o) for o in outs]
        split = [np.split(o, n_cores, axis=0) for o in outs]
        return [{name: split[i][c] for i, name in enumerate(out_names)}
                for c in range(n_cores)]

    def put_inputs(in_maps):
        cat = [np.concatenate([np.asarray(in_maps[c][nm])
                               for c in range(n_cores)], axis=0)
               for nm in in_names]
        return [jax.device_put(a) for a in cat]

    _CACHE["runner"] = (run, put_inputs, fn, n_params, n_outs)
    return _CACHE["runner"]


def _sim_fallback(nc, in_maps):
    """Correctness fallback if the PJRT/hardware path errors: run each
    core's shard through CoreSim."""
    from concourse.bass_interp import CoreSim
    results = []
    for m in in_maps:
        sim = CoreSim(nc, require_finite=False)
        for k, v in m.items():
            sim.tensor(k)[:] = v
        sim.simulate(check_with_hw=False)
        results.append({"y": np.array(sim.tensor("y"))})
    return results


def kernel(**inputs):
    in_maps = _prep_inputs(inputs)
    try:
        run, put_inputs, _, _, _ = _get_runner()
        results = run(in_maps)
    except Exception as e:
        sys.stderr.write(f"kernel: hardware path failed ({e!r}); "
                         "falling back to CoreSim\n")
        results = _sim_fallback(_CACHE["nc"], in_maps)
    out = np.empty((B, C, N), np.float32)
    for r in range(8):
        b = r // 4
        qs = slice((r % 4) * NQ, (r % 4 + 1) * NQ)
        out[b, :, qs] = np.asarray(results[r]["y"]).reshape(C, NQ)
    return out


# revision 47
# speedup vs baseline: 1.0000x; 1.0000x over previous
"""Trainium2 Bass kernel for GNN message-passing attention block.

Sharding: core c handles batch c//4 and query block (c%4)*256..+256.
Each core computes GroupNorm + K/V projections over its batch's full
n=1024 nodes (attend_idx may reference any node), but only its own 256
queries' Q/scores/softmax/AV/output.

The sparse gather/scatter collapses into a dense multiplicity mask
M[j,q] (host-precomputed): duplicate neighbor indices share a score, so
their combined softmax weight is M * exp(s) / Z.  Z comes free from an
appended ones-column in the V^T stationary.  The K bias is dropped
(softmax-invariant); the Q bias rides the PSUM->SBUF Identity cast.

Schedule notes (from NTFF trace iterations; baseline 126us -> ~75us):
- DMA issues cost ~600ns of issuing-engine time each: tensors are
  consolidated to one DMA per tensor (host pre-transposes to
  partition-major), split across scalar/gpsimd/sync queues by need-time.
- exp/mask work in [128,1024] ops (jc-pairs) to amortize fixed costs;
  scores interleave K/V-projection and AV fillers so the PE (the pacing
  engine: ~230 matmuls at a power-throttled ~1.2-1.8GHz) never idles
  while ACT streams exps.
- AV groups 0/1 run as fillers inside the last score block, so only
  group 3's AV+normalize trails the exp stream.
- All PSUM evacuation (K/V/Q casts, Z rows, osb) must stay off GPSIMD
  (no PSUM access) and off ACT mid-stream (delays exp); DVE carries it.
"""

import sys

if "/opt/trn_rl_repo" not in sys.path:
    sys.path.insert(0, "/opt/trn_rl_repo")

import numpy as np

import concourse.bacc as bacc
import concourse.mybir as mybir
import concourse.tile as tile
from concourse import bass_utils
from contextlib import ExitStack

F32 = mybir.dt.float32
BF16 = mybir.dt.bfloat16
AF = mybir.ActivationFunctionType
ALU = mybir.AluOpType

B, C, N, K, H, DH, NG = 2, 512, 1024, 64, 8, 64, 32
NQ = 256            # queries per core
NCHUNK = C // 128   # 4 channel chunks
NJC = 8             # key-node chunks of 128
NJP = 4             # jc pairs
EPS = 1e-6

_CACHE = {}


def _emit(tc, nc, t):
    ctx = t["ctx"]
    P = 128

    wpool = ctx.enter_context(tc.tile_pool(name="weights", bufs=1))
    cpool = ctx.enter_context(tc.tile_pool(name="consts", bufs=1))
    xpool = ctx.enter_context(tc.tile_pool(name="x", bufs=1))
    hpool = ctx.enter_context(tc.tile_pool(name="h", bufs=1))
    kvpool = ctx.enter_context(tc.tile_pool(name="kv", bufs=1))
    apool = ctx.enter_context(tc.tile_pool(name="attn", bufs=1))
    spool = ctx.enter_context(tc.tile_pool(name="scratch", bufs=2))
    smallp = ctx.enter_context(tc.tile_pool(name="small", bufs=1))
    opool = ctx.enter_context(tc.tile_pool(name="out", bufs=1))
    ypool = ctx.enter_context(tc.tile_pool(name="yp", bufs=2))
    # PSUM: pp 2x[128,512] + ps 2x[128,1024] + po 2x[65,512] = 8 banks
    pp = ctx.enter_context(tc.tile_pool(name="psum", bufs=2, space="PSUM"))
    ps_pool = ctx.enter_context(tc.tile_pool(name="psum_s", bufs=2, space="PSUM"))
    po_pool = ctx.enter_context(tc.tile_pool(name="psum_o", bufs=2, space="PSUM"))

    # ---- DMA issues, one per tensor, ordered by first-use ----
    gbwd = cpool.tile([8, P], F32)
    consts = cpool.tile([P, 12], F32)      # gamma|beta|bo_eff (4 cols each)
    bq4 = cpool.tile([P, 4], F32)
    gfwd = cpool.tile([P, 8], F32)
    nc.scalar.dma_start(gbwd[:], t["gbwd"][:, :])     # first: feeds PE warmup
    nc.scalar.dma_start(consts[:], t["consts"][:, :])
    nc.scalar.dma_start(bq4[:], t["bq4"][:, :])
    nc.scalar.dma_start(gfwd[:], t["gfwd"][:, :])

    xsb = [xpool.tile([P, N], BF16, tag=f"x{m}", name=f"x{m}") for m in range(NCHUNK)]
    xqs = xpool.tile([P, NCHUNK, NQ], F32)
    wq = wpool.tile([P, NCHUNK, 512], BF16, tag="wq", name="wq")
    wk = wpool.tile([P, NCHUNK, 512], BF16, tag="wk", name="wk")
    wv = wpool.tile([P, NCHUNK, 512], BF16, tag="wv", name="wv")
    wo = wpool.tile([P, NCHUNK, 512], BF16, tag="wo", name="wo")
    msb = cpool.tile([P, NJP, 2, 512], BF16, tag="msb", name="msb")

    for m in range(NCHUNK):
        nc.gpsimd.dma_start(xsb[m][:], t["xb"][m, :, :])
    nc.gpsimd.dma_start(wq[:], t["wqT"][:, :, :])
    nc.sync.dma_start(xqs[:], t["xq"][:, :, :])
    nc.sync.dma_start(wk[:], t["wkT"][:, :, :])
    xqb = xpool.tile([P, NCHUNK, NQ + 1], BF16, tag="xqb", name="xqb")
    nc.sync.dma_start(xqb[:, :, 0:NQ], t["xqb"][:, :, :])
    nc.gpsimd.dma_start(wv[:], t["wvT"][:, :, :])
    nc.gpsimd.dma_start(wo[:], t["woT"][:, :, :])
    nc.gpsimd.dma_start(msb[:], t["mmask"][:, :, :, :])

    ones64 = cpool.tile([1, DH], BF16)
    nc.gpsimd.memset(ones64[:], 1.0)
    expwarm = cpool.tile([1, 2], F32)
    nc.gpsimd.memset(expwarm[:], 0.0)

    # ---- GroupNorm statistics (per-partition bn_stats -> PE group aggr) ----
    ssq = smallp.tile([P, 2, 4], F32)    # [., 0, m]=mean_p, [., 1, m]=E[x^2]_p
    sqt = smallp.tile([P, 4], F32)
    bnraw = smallp.tile([P, 2, 6], F32)
    for m in range(NCHUNK):
        for tblk in range(2):
            nc.vector.bn_stats(bnraw[:, tblk, :],
                               xsb[m][:, tblk * 512:(tblk + 1) * 512])
        # (mean, var) -> strided cols (m, 4+m) of ssq
        nc.vector.bn_aggr(ssq[:, :, m], bnraw[:])
    # E[x^2] = var + mean^2 for all four chunks at once
    nc.vector.tensor_tensor(sqt[:], ssq[:, 0, :], ssq[:, 0, :], ALU.mult)
    nc.vector.tensor_tensor(ssq[:, 1, :], sqt[:], ssq[:, 1, :], ALU.add)
    gs = pp.tile([8, 8], F32, tag="mm", name="gs")
    nc.tensor.matmul(gs[:], gfwd[:], ssq[:], start=True, stop=True)
    mu = smallp.tile([8, 8], F32)
    nc.scalar.activation(mu[:], gs[:], AF.Copy, scale=1.0 / 16.0)
    var = smallp.tile([8, 4], F32)
    nc.vector.tensor_tensor(var[:], mu[:, 0:4], mu[:, 0:4], ALU.mult)
    nc.vector.tensor_tensor(var[:], mu[:, 4:8], var[:], ALU.subtract)
    sd = smallp.tile([8, 4], F32)
    nc.vector.tensor_scalar_add(sd[:], var[:], EPS)
    sdq = smallp.tile([8, 4], F32)
    nc.scalar.activation(sdq[:], sd[:], AF.Sqrt)
    # Warm the exp act-table now: exp_and_others also holds identity/copy,
    # so the Identity/Copy casts below cause no further table loads.
    nc.scalar.activation(expwarm[:], expwarm[:], AF.Exp)
    rs = smallp.tile([8, 4], F32)
    nc.vector.reciprocal(rs[:], sdq[:])
    bc = pp.tile([P, 8], F32, tag="mm", name="bc")
    nc.tensor.matmul(bc[:, 0:4], gbwd[:], mu[:, 0:4], start=True, stop=True)
    nc.tensor.matmul(bc[:, 4:8], gbwd[:], rs[:], start=True, stop=True)
    ga = smallp.tile([P, 4], F32)
    gb = smallp.tile([P, 4], F32)
    nc.vector.tensor_tensor(ga[:], consts[:, 0:4], bc[:, 4:8], ALU.mult)
    nc.vector.tensor_tensor(gb[:], bc[:, 0:4], ga[:], ALU.mult)
    nc.vector.tensor_tensor(gb[:], consts[:, 4:8], gb[:], ALU.subtract)

    # ---- Fold GN into the K/Q weights: k = Wk(ga*x + gb) = (Wk diag(ga)) x
    # + Wk gb.  The K bias term is softmax-invariant (drops); Q's comes out
    # of a gb column appended to the moving operand.  K/Q then consume raw
    # x and skip the h stage entirely; V keeps the h path (x is its
    # stationary operand, so the gb term is not a per-channel constant
    # there) ----
    for m in range(NCHUNK):
        nc.vector.tensor_scalar(wk[:, m, :], wk[:, m, :], ga[:, m:m + 1],
                                None, ALU.mult)
    for m in range(NCHUNK):
        nc.vector.tensor_copy(xqb[:, m, NQ:NQ + 1], gb[:, m:m + 1])
        nc.vector.tensor_scalar(wq[:, m, :], wq[:, m, :], ga[:, m:m + 1],
                                None, ALU.mult)
    hsb = [hpool.tile([P, N], BF16, tag=f"h{m}", name=f"h{m}") for m in range(NCHUNK)]

    # ---- K projection (bias dropped), V projection.  Odd head halves get
    # partition-0 copies via SBUF DMA (matmul operands at partition offset
    # 64 crash hardware); K group 0 runs first so its odd-half DMA hides
    # under the Q projection ----
    qsb = kvpool.tile([P, NCHUNK, NQ], BF16, tag="q", name="q")
    qOd = kvpool.tile([64, NCHUNK, NQ], BF16, tag="qO", name="qO")
    ksb = kvpool.tile([P, NCHUNK, N], BF16, tag="k", name="k")
    kOd = kvpool.tile([64, NCHUNK, N], BF16, tag="kO", name="kO")
    vT = [kvpool.tile([P, H, DH + 1], BF16, tag=f"vT{jc}", name=f"vT{jc}")
          for jc in range(NJC)]

    def k_nt(g, nt):
        osl = slice(g * 128, (g + 1) * 128)
        nsl = slice(nt * 512, (nt + 1) * 512)
        pk = pp.tile([P, 512], F32, tag="mm", name=f"pk{g}_{nt}")
        for ki in range(NCHUNK):
            nc.tensor.matmul(pk[:], wk[:, ki, osl], xsb[ki][:, nsl],
                             start=(ki == 0), stop=(ki == NCHUNK - 1))
        nc.vector.tensor_copy(ksb[:, g, nsl], pk[:])
        if nt == 1:
            nc.sync.dma_start(kOd[:, g, :], ksb[64:128, g, :])

    def v_jc(jc):
        jsl = slice(jc * 128, (jc + 1) * 128)
        pv = pp.tile([P, C], F32, tag="mm", name=f"pv{jc}")
        for ki in range(NCHUNK):
            nc.tensor.matmul(pv[:], hsb[ki][:, jsl], wv[:, ki, :],
                             start=(ki == 0), stop=(ki == NCHUNK - 1))
        nc.vector.tensor_copy(vT[jc][:, :, 0:DH],
                              pv[:].rearrange("p (h d) -> p h d", h=H))
        nc.gpsimd.memset(vT[jc][:, :, DH:DH + 1], 1.0)

    k_nt(0, 0)
    k_nt(0, 1)

    # ---- Q projection on raw x; pq column NQ accumulates Wq@gb, which
    # plus bq becomes the per-channel bias of the Identity cast ----
    bqe = smallp.tile([P, 4], F32)
    for g in range(NCHUNK):
        osl = slice(g * 128, (g + 1) * 128)
        pq = pp.tile([P, NQ + 1], F32, tag="mm", name=f"pq{g}")
        for ki in range(NCHUNK):
            nc.tensor.matmul(pq[:], wq[:, ki, osl], xqb[:, ki, :],
                             start=(ki == 0), stop=(ki == NCHUNK - 1))
        nc.vector.tensor_tensor(bqe[:, g:g + 1], pq[:, NQ:NQ + 1],
                                bq4[:, g:g + 1], ALU.add)
        nc.scalar.activation(qsb[:, g, :], pq[:, 0:NQ], AF.Identity,
                             bias=bqe[:, g:g + 1])
        if g == 0:
            nc.sync.dma_start(qOd[:, 0, :], qsb[64:128, 0, :])
    nc.sync.dma_start(qOd[:, 1:4, :], qsb[64:128, 1:4, :])

    # h (for the V projection only) after the critical K0/Q evacuations in
    # the DVE queue; first V filler runs in the g0 score block
    for m in range(NCHUNK):
        nc.vector.tensor_scalar(hsb[m][:], xsb[m][:], ga[:, m:m + 1],
                                gb[:, m:m + 1], ALU.mult, ALU.add)

    # ---- scores -> exp -> mask at jc-pair granularity, with fillers ----
    asb = [apool.tile([P, 2, NCHUNK, 512], BF16, tag=f"a{jp}", name=f"a{jp}")
           for jp in range(NJP)]

    def scores_jp(g, jp):
        ps = ps_pool.tile([P, 2, 2, NQ], F32, tag="ps", name=f"ps{g}_{jp}")
        for jcin in range(2):
            jc = 2 * jp + jcin
            jsl = slice(jc * 128, (jc + 1) * 128)
            nc.tensor.matmul(ps[:, jcin, 0, :], ksb[0:64, g, jsl],
                             qsb[0:64, g, :], start=True, stop=True)
            nc.tensor.matmul(ps[:, jcin, 1, :], kOd[:, g, jsl],
                             qOd[:, g, :], start=True, stop=True)
        dst = asb[jp][:, :, g, :]
        nc.scalar.activation(dst, ps[:], AF.Exp)
        nc.vector.tensor_tensor(dst, dst, msb[:, jp, :, :], ALU.mult)

    po = [None] * NCHUNK
    zsb = [None] * NCHUNK
    zbci = [None] * NCHUNK

    def av_g(g):
        po[g] = po_pool.tile([DH + 1, 2 * NQ], F32, tag="po", name=f"po{g}")
        for h2 in range(2):
            h = 2 * g + h2
            blk = slice(h2 * NQ, (h2 + 1) * NQ)
            for jc in range(NJC):
                nc.tensor.matmul(po[g][:, blk], vT[jc][:, h, :],
                                 asb[jc // 2][:, jc % 2, g,
                                              h2 * NQ:(h2 + 1) * NQ],
                                 start=(jc == 0), stop=(jc == NJC - 1))

    def zsb_g(g):
        zsb[g] = spool.tile([1, 2 * NQ], BF16, tag="zsb", name=f"zsb{g}")
        if g == NCHUNK - 1:
            nc.vector.tensor_copy(zsb[g][:], po[g][DH:DH + 1, :])
        else:
            nc.scalar.activation(zsb[g][:], po[g][DH:DH + 1, :], AF.Copy)

    def pz_g(g):
        pz = pp.tile([DH, 2 * NQ], F32, tag="mm", name=f"pz{g}")
        nc.tensor.matmul(pz[:], ones64[:], zsb[g][:], start=True, stop=True)
        return pz

    osb = [opool.tile([P, NQ], BF16, tag=f"o{g}", name=f"o{g}") for g in range(NCHUNK)]

    def norm_g(g, pz):
        zbci[g] = spool.tile([DH, 2 * NQ], F32, tag="zbci", name=f"zbci{g}")
        nc.vector.reciprocal_approx_fast(zbci[g][:], pz[:])
        nc.vector.tensor_tensor(osb[g][0:64, :], po[g][0:DH, 0:NQ],
                                zbci[g][:, 0:NQ], ALU.mult)
        nc.vector.tensor_tensor(osb[g][64:128, :], po[g][0:DH, NQ:2 * NQ],
                                zbci[g][:, NQ:2 * NQ], ALU.mult)

    def av_z(g):
        av_g(g)
        zsb_g(g)

    fillers = {0: [lambda: k_nt(1, 0), lambda: k_nt(1, 1),
                   lambda: v_jc(0), lambda: v_jc(1)],
               1: [lambda: k_nt(2, 0), lambda: k_nt(2, 1),
                   lambda: v_jc(2), lambda: v_jc(3)],
               2: [lambda: k_nt(3, 0), lambda: k_nt(3, 1),
                   lambda: v_jc(4), lambda: v_jc(5)],
               3: [lambda: v_jc(6), lambda: v_jc(7),
                   lambda: av_z(0), lambda: av_z(1)]}
    for g in range(NCHUNK):
        todo = list(fillers[g])
        for jp in range(NJP):
            scores_jp(g, jp)
            if jp == 0 and todo:
                todo.pop(0)()
            if jp < NJP - 1 and todo:
                todo.pop(0)()
        while todo:
            todo.pop(0)()

    av_g(2)
    zsb_g(2)
    norm_g(0, pz_g(0))
    norm_g(1, pz_g(1))
    norm_g(2, pz_g(2))

    # ---- output projection + bias (bo_eff folds the V bias) + residual.
    # Chunks mo 0/1 pre-accumulate head-pairs 0-2 in the two pp banks while
    # group 3 is still normalizing; only the last contraction chunk trails
    # osb[3]. ----
    pys = {}
    for mo in (0,):
        osl = slice(mo * 128, (mo + 1) * 128)
        py = pp.tile([P, NQ], F32, tag="mm", name=f"py{mo}")
        for ki in range(NCHUNK - 1):
            nc.tensor.matmul(py[:], wo[:, ki, osl], osb[ki][:],
                             start=(ki == 0), stop=False)
        pys[mo] = py

    av_g(3)
    zsb_g(3)
    norm_g(3, pz_g(3))

    for mo in range(NCHUNK):
        osl = slice(mo * 128, (mo + 1) * 128)
        if mo in pys:
            py = pys[mo]
            nc.tensor.matmul(py[:], wo[:, NCHUNK - 1, osl], osb[NCHUNK - 1][:],
                             start=False, stop=True)
        else:
            py = pp.tile([P, NQ], F32, tag="mm", name=f"py{mo}")
            for ki in range(NCHUNK):
                nc.tensor.matmul(py[:], wo[:, ki, osl], osb[ki][:],
                                 start=(ki == 0), stop=(ki == NCHUNK - 1))
        ysb = ypool.tile([P, NQ], F32, tag="y", name=f"y{mo}")
        nc.vector.scalar_tensor_tensor(ysb[:], py[:], consts[:, 8 + mo:9 + mo],
                                       xqs[:, mo, :], ALU.add, ALU.add)
        nc.sync.dma_start(t["y"][mo, :, :], ysb[:])


def _build():
    nc = bacc.Bacc("TRN2", target_bir_lowering=False, debug=False, num_devices=8)
    t = {}
    t["xb"] = nc.dram_tensor("xb", [NCHUNK, 128, N], BF16, kind="ExternalInput").ap()
    t["xq"] = nc.dram_tensor("xq", [128, NCHUNK, NQ], F32, kind="ExternalInput").ap()
    t["xqb"] = nc.dram_tensor("xqb", [128, NCHUNK, NQ], BF16, kind="ExternalInput").ap()
    t["mmask"] = nc.dram_tensor("mmask", [128, NJP, 2, 512], BF16,
                                kind="ExternalInput").ap()
    for w in ("wqT", "wkT", "wvT", "woT"):
        t[w] = nc.dram_tensor(w, [128, NCHUNK, 512], BF16, kind="ExternalInput").ap()
    t["consts"] = nc.dram_tensor("consts", [128, 12], F32, kind="ExternalInput").ap()
    t["bq4"] = nc.dram_tensor("bq4", [128, 4], F32, kind="ExternalInput").ap()
    t["gfwd"] = nc.dram_tensor("gfwd", [128, 8], F32, kind="ExternalInput").ap()
    t["gbwd"] = nc.dram_tensor("gbwd", [8, 128], F32, kind="ExternalInput").ap()
    t["y"] = nc.dram_tensor("y", [NCHUNK, 128, NQ], F32, kind="ExternalOutput").ap()
    with tile.TileContext(nc) as tc, ExitStack() as ctx:
        t["ctx"] = ctx
        _emit(tc, nc, t)
    nc.compile()
    return nc


def _prep_inputs(inputs):
    x = np.ascontiguousarray(np.asarray(inputs["x"], dtype=np.float32))
    idx = np.asarray(inputs["attend_idx"]).astype(np.int64)
    vm = np.asarray(inputs["valid_mask"]).astype(np.float32)
    wq = np.asarray(inputs["wq"], dtype=np.float32)
    wk = np.asarray(inputs["wk"], dtype=np.float32)
    wv = np.asarray(inputs["wv"], dtype=np.float32)
    wo = np.asarray(inputs["wo"], dtype=np.float32)
    bq = np.asarray(inputs["bq"], dtype=np.float32)
    bv = np.asarray(inputs["bv"], dtype=np.float32)
    bo = np.asarray(inputs["bo"], dtype=np.float32)
    gamma = np.asarray(inputs["gn_gamma"], dtype=np.float32)
    beta = np.asarray(inputs["gn_beta"], dtype=np.float32)

    cols = np.arange(C)
    perm = (cols % DH) * H + cols // DH   # wo_perm[:, h*64+d] = wo[:, d*8+h]
    wo_perm = wo[:, perm]
    bo_eff = bo + wo_perm @ bv

    def colmajor(v):
        return np.ascontiguousarray(v.reshape(NCHUNK, 128).T)

    def wchunks(wT):
        # [C, C] -> [128, NCHUNK, 512] so one contiguous DMA fills the tile
        return np.ascontiguousarray(wT.reshape(NCHUNK, 128, C).transpose(1, 0, 2))

    consts = np.concatenate([colmajor(v) for v in (gamma, beta, bo_eff)], axis=1)
    gfwd = np.zeros((128, 8), np.float32)
    gfwd[np.arange(128), np.arange(128) // 16] = 1.0
    gbwd = np.ascontiguousarray(gfwd.T)

    from ml_dtypes import bfloat16
    x_bf = x.astype(bfloat16)
    shared = {
        "wqT": wchunks(wq.T).astype(bfloat16),
        "wkT": wchunks(wk.T).astype(bfloat16),
        "wvT": wchunks(wv.T).astype(bfloat16),
        "woT": wchunks(wo_perm.T).astype(bfloat16),
        "consts": np.ascontiguousarray(consts),
        "bq4": np.ascontiguousarray(colmajor(bq)),
        "gfwd": gfwd,
        "gbwd": gbwd,
    }
    in_maps = []
    for r in range(8):
        b = r // 4
        qs = slice((r % 4) * NQ, (r % 4 + 1) * NQ)
        Mr = np.zeros((N, NQ), np.float32)
        np.add.at(Mr, (idx[qs].ravel(), np.repeat(np.arange(NQ), K)),
                  vm[qs].ravel())
        M2 = Mr.reshape(NJC, 128, NQ)
        # [128, jp, jc-in-pair(2), dup(2)*256] head-duplicated mask blocks
        mk = np.empty((128, NJP, 2, 2, NQ), np.float32)
        for jp in range(NJP):
            for jcin in range(2):
                mk[:, jp, jcin, 0, :] = M2[2 * jp + jcin]
                mk[:, jp, jcin, 1, :] = M2[2 * jp + jcin]
        m = dict(shared)
        m["mmask"] = np.ascontiguousarray(
            mk.reshape(128, NJP, 2, 512)).astype(bfloat16)
        m["xb"] = np.ascontiguousarray(x_bf[b].reshape(NCHUNK, 128, N))
        m["xq"] = np.ascontiguousarray(
            x[b, :, qs].reshape(NCHUNK, 128, NQ).transpose(1, 0, 2))
        m["xqb"] = np.ascontiguousarray(
            x_bf[b][:, qs].reshape(NCHUNK, 128, NQ).transpose(1, 0, 2))
        in_maps.append(m)
    return in_maps


def _get_runner(n_cores=8):
    """Build (once) a cached jitted SPMD executor mirroring
    bass2jax.run_bass_via_pjrt, so repeated calls don't re-trace."""
    if "runner" in _CACHE:
        return _CACHE["runner"]
    if "nc" not in _CACHE:
        _CACHE["nc"] = _build()
    nc = _CACHE["nc"]
    import jax
    from jax.sharding import Mesh, PartitionSpec
    from jax.experimental.shard_map import shard_map
    from concourse import bass2jax
    import concourse.mybir as _mybir

    bass2jax.install_neuronx_cc_hook()
    part_name = nc.partition_id_tensor.name if nc.partition_id_tensor else None
    in_names, out_names, out_avals, zero_outs = [], [], [], []
    for alloc in nc.m.functions[0].allocations:
        if not isinstance(alloc, _mybir.MemoryLocationSet):
            continue
        name = alloc.memorylocations[0].name
        if alloc.kind == "ExternalInput":
            if name != part_name:
                in_names.append(name)
        elif alloc.kind == "ExternalOutput":
            shape = tuple(alloc.tensor_shape)
            dtype = _mybir.dt.np(alloc.dtype)
            out_names.append(name)
            out_avals.append(jax.core.ShapedArray(shape, dtype))
            zero_outs.append(np.zeros(shape, dtype))
    n_params = len(in_names)
    n_outs = len(out_avals)
    all_names = in_names + out_names
    if part_name is not None:
        all_names = all_names + [part_name]
    donate = tuple(range(n_params, n_params + n_outs))

    def _body(*args):
        operands = list(args)
        if part_name is not None:
            operands.append(bass2jax.partition_id_tensor())
        outs = bass2jax._bass_exec_p.bind(
            *operands,
            out_avals=tuple(out_avals),
            in_names=tuple(all_names),
            out_names=tuple(out_names),
            lowering_input_output_aliases=(),
            sim_require_finite=True,
            sim_require_nnan=True,
            nc=nc,
        )
        return tuple(outs)

    devices = jax.devices()[:n_cores]
    mesh = Mesh(np.asarray(devices), ("core",))
    fn = jax.jit(
        shard_map(_body, mesh=mesh,
                  in_specs=(PartitionSpec("core"),) * (n_params + n_outs),
                  out_specs=(PartitionSpec("core"),) * n_outs,
                  check_rep=False),
        donate_argnums=donate, keep_unused=True)

    def run(in_maps, device_inputs=None):
        if device_inputs is None:
            device_inputs = put_inputs(in_maps)
        zo = [np.concatenate([np.zeros_like(z)] * n_cores, axis=0)
              for z in zero_outs]
        outs = fn(*device_inputs, *zo)
        outs = [np.asarray(o) for o in outs]
        split = [np.split(o, n_cores, axis=0) for o in outs]
        return [{name: split[i][c] for i, name in enumerate(out_names)}
                for c in range(n_cores)]

    def put_inputs(in_maps):
        cat = [np.concatenate([np.asarray(in_maps[c][nm])
                               for c in range(n_cores)], axis=0)
               for nm in in_names]
        return [jax.device_put(a) for a in cat]

    _CACHE["runner"] = (run, put_inputs, fn, n_params, n_outs)
    return _CACHE["runner"]


def _sim_fallback(nc, in_maps):
    """Correctness fallback if the PJRT/hardware path errors: run each
    core's shard through CoreSim."""
    from concourse.bass_interp import CoreSim
    results = []
    for m in in_maps:
        sim = CoreSim(nc, require_finite=False)
        for k, v in m.items():
            sim.tensor(k)[:] = v
        sim.simulate(check_with_hw=False)
        results.append({"y": np.array(sim.tensor("y"))})
    return results


def kernel(**inputs):
    in_maps = _prep_inputs(inputs)
    try:
        run, put_inputs, _, _, _ = _get_runner()
        results = run(in_maps)
    except Exception as e:
        sys.stderr.write(f"kernel: hardware path failed ({e!r}); "
                         "falling back to CoreSim\n")
        results = _sim_fallback(_CACHE["nc"], in_maps)
    out = np.empty((B, C, N), np.float32)
    for r in range(8):
        b = r // 4
        qs = slice((r % 4) * NQ, (r % 4 + 1) * NQ)
        out[b, :, qs] = np.asarray(results[r]["y"]).reshape(C, NQ)
    return out
